# revision 11
# baseline (speedup 1.0000x reference)
"""Per-neuron grouped MLP (conv-style) kernel for Trainium2, 8 NeuronCores.

Math (per group d):  h = x[:, d, :] @ W1[d].T + b1[d]; g = gelu(h); out[:, d] = g @ W2[d] + b2[d]
  x: [B=512, D=2048, M=128], W1: [D, H=128, M], b1: [D, H], W2: [D, H], b2: [D]

Strategy:
  - Shard on D: each of 8 cores owns D_LOC = 256 independent per-neuron MLPs.
  - Host pre-transposes so every DMA is contiguous:
      xT[d, m, b], W1T[d, m, h], W2T[h, d], b1T[h, d]
  - Inputs cast to fp16 on host (PSUM accumulation stays fp32): 4x PE stream
    rate vs fp32 and half the DMA bytes, with ~1e-3 worst-case output error.
  - Per quad of 4 d's on-chip (contraction dims on partitions):
      psum1[H, 2B] = W1T[d].T @ xT[d]          (N=512 matmuls, 2 per psum tile)
      g[H, 4B]     = gelu(psum1 + b1[d])       (ScalarE, exact-erf Gelu)
      psum2[128,B] = 4 packed matmuls W2T[:,d].T @ g_d, tile_position=(0,32j)
                     -> rows {0,32,64,96}
      o_sb         = DVE full-tile copy of psum2 (only 4 rows carry data)
      outT[d:d+4] <- strided-partition DMA of o_sb rows {0,32,64,96}
  - b2 added on host (it is outside the nonlinearity).
"""

import numpy as np

B, D, M, H = 512, 2048, 128, 128
N_CORES = 8
D_LOC = D // N_CORES  # 256
QUAD = 4     # d's per MM2 packing group
PAIR = 2     # d's per psum1/ACT batch
SUPER = 16   # d's per super-group: one x DMA, one w1 DMA, one out DMA
# Within a super-group [D0, D0+16), quad c (c=0..3) handles d = D0 + 4j + c
# (j=0..3); MM2 j lands on psum row 32j, so out rows {D0..D0+15} are exactly
# o_sb[0::32, c, :] in (row, quad, b) iteration order -> single strided DMA.

PRECISION = "fp16"  # "fp16" | "fp32"

_NC_CACHE = {}


def build_nc(bias_mode: bool, prec: str = PRECISION, reps: int = 1):
    """Build + compile the Bass module (shared SPMD program for all 8 cores).

    reps>1 replicates the whole body (same in/out DRAM) for benchmarking:
    one NEFF execution then runs the kernel `reps` times back-to-back."""
    key = (bias_mode, prec, reps)
    if key in _NC_CACHE:
        return _NC_CACHE[key]

    import concourse.bacc as bacc
    import concourse.mybir as mybir
    import concourse.tile as tile

    f32 = mybir.dt.float32
    dt = f32 if prec == "fp32" else mybir.dt.float16
    GELU = mybir.ActivationFunctionType.Gelu

    nc = bacc.Bacc("TRN2", target_bir_lowering=False, debug=False, num_devices=N_CORES)
    xT = nc.dram_tensor("xT", [D_LOC, M, B], dt, kind="ExternalInput").ap()
    w1T = nc.dram_tensor("w1T", [D_LOC, M, H], dt, kind="ExternalInput").ap()
    w2T = nc.dram_tensor("w2T", [H, D_LOC], dt, kind="ExternalInput").ap()
    b1T = nc.dram_tensor("b1T", [H, D_LOC], f32, kind="ExternalInput").ap()
    outT = nc.dram_tensor("outT", [D_LOC, B], f32, kind="ExternalOutput").ap()

    with (
        tile.TileContext(nc) as tc,
        tc.tile_pool(name="singles", bufs=1) as singles,
        tc.tile_pool(name="xp", bufs=3) as xp,
        tc.tile_pool(name="wp", bufs=2) as wp,
        tc.tile_pool(name="gp", bufs=4) as gp,
        tc.tile_pool(name="op", bufs=4) as op_pool,
        tc.tile_pool(name="ps1", bufs=2, space="PSUM") as ps1,
        tc.tile_pool(name="ps2", bufs=2, space="PSUM") as ps2,
    ):
        w2_sb = singles.tile([H, D_LOC], dt)
        nc.sync.dma_start(out=w2_sb[:], in_=w2T[:])
        b1_sb = None
        if bias_mode:
            b1_sb = singles.tile([H, D_LOC], f32)
            nc.sync.dma_start(out=b1_sb[:], in_=b1T[:])

        for _rep in range(reps):
            _body_loop(nc, tc, bias_mode, dt, f32, GELU,
                       xT, w1T, outT, w2_sb, b1_sb,
                       xp, wp, gp, op_pool, ps1, ps2)

    nc.compile()
    _NC_CACHE[key] = nc
    return nc


def _body_loop(nc, tc, bias_mode, dt, f32, GELU, xT, w1T, outT, w2_sb, b1_sb,
               xp, wp, gp, op_pool, ps1, ps2):
        for sg in range(D_LOC // SUPER):
            D0 = sg * SUPER
            x_sb = xp.tile([M, SUPER, B], dt)
            nc.sync.dma_start(
                out=x_sb[:],
                in_=xT[D0 : D0 + SUPER].rearrange("d m b -> m d b"),
            )
            w1_sb = wp.tile([M, SUPER, H], dt)
            nc.scalar.dma_start(
                out=w1_sb[:],
                in_=w1T[D0 : D0 + SUPER].rearrange("d m h -> m d h"),
            )
            o_sb = op_pool.tile([128, SUPER // QUAD, B], f32)
            for c in range(SUPER // QUAD):
                # quad c handles d = D0 + 4j + c, j = 0..3
                g_sb = gp.tile([H, QUAD * B], dt)
                for pr in range(QUAD // PAIR):
                    p1 = ps1.tile([H, PAIR * B], f32)
                    for j in range(PAIR):
                        jj = pr * PAIR + j
                        nc.tensor.matmul(
                            p1[:, j * B : (j + 1) * B],
                            lhsT=w1_sb[:, 4 * jj + c, :],
                            rhs=x_sb[:, 4 * jj + c, :],
                            start=True,
                            stop=True,
                        )
                    if bias_mode:
                        for j in range(PAIR):
                            jj = pr * PAIR + j
                            dd = D0 + 4 * jj + c
                            nc.scalar.activation(
                                g_sb[:, jj * B : (jj + 1) * B],
                                p1[:, j * B : (j + 1) * B],
                                GELU,
                                bias=b1_sb[:, dd : dd + 1],
                            )
                    else:
                        nc.scalar.activation(
                            g_sb[:, pr * PAIR * B : (pr + 1) * PAIR * B], p1[:], GELU
                        )
                p2 = ps2.tile([128, B], f32)
                for j in range(QUAD):
                    dd = D0 + 4 * j + c
                    nc.tensor.matmul(
                        p2[32 * j : 32 * j + 1, :],
                        lhsT=w2_sb[:, dd : dd + 1],
                        rhs=g_sb[:, j * B : (j + 1) * B],
                        start=True,
                        stop=True,
                        tile_position=(0, 32 * j),
                    )
                nc.vector.tensor_copy(o_sb[:, c, :], p2[:])
            nc.gpsimd.dma_start(
                out=outT[D0 : D0 + SUPER, :], in_=o_sb[0::32, :, :]
            )


def prepare_in_maps(x, W1, b1, W2, prec: str = PRECISION):
    """Host-side shard + transpose. Returns list of 8 per-core input dicts."""
    np_dt = np.float32 if prec == "fp32" else np.float16
    x = np.asarray(x, dtype=np.float32)
    W1 = np.asarray(W1, dtype=np.float32)
    b1 = np.asarray(b1, dtype=np.float32)
    W2 = np.asarray(W2, dtype=np.float32)

    in_maps = []
    for k in range(N_CORES):
        sl = slice(k * D_LOC, (k + 1) * D_LOC)
        # [B, D_LOC, M] -> [D_LOC, M, B]; see kernel docstring for why.
        xT_k = np.ascontiguousarray(x[:, sl, :].transpose(1, 2, 0), dtype=np_dt)
        w1T_k = np.ascontiguousarray(W1[sl].transpose(0, 2, 1), dtype=np_dt)
        w2T_k = np.ascontiguousarray(W2[sl].T, dtype=np_dt)
        b1T_k = np.ascontiguousarray(b1[sl].T, dtype=np.float32)
        in_maps.append({"xT": xT_k, "w1T": w1T_k, "w2T": w2T_k, "b1T": b1T_k})
    return in_maps


def assemble_output(results, b2):
    outT_full = np.concatenate([r["outT"] for r in results], axis=0)  # [D, B]
    out = outT_full.T  # [B, D]
    b2 = np.asarray(b2, dtype=np.float32)
    if np.any(b2):
        out = out + b2[None, :]
    return np.ascontiguousarray(out)


def kernel(pre_activation_history, W1, b1, W2, b2):
    from concourse.bass_utils import run_bass_kernel_spmd

    b1 = np.asarray(b1, dtype=np.float32)
    bias_mode = bool(np.any(b1))
    nc = build_nc(bias_mode)
    in_maps = prepare_in_maps(pre_activation_history, W1, b1, W2)
    res = run_bass_kernel_spmd(nc, in_maps, core_ids=list(range(N_CORES)))
    return assemble_output(res.results, b2)


# revision 17
# speedup vs baseline: 2.6086x; 2.6086x over previous
"""Per-neuron grouped MLP (conv-style) kernel for Trainium2, 8 NeuronCores.

Math (per group d):  h = x[:, d, :] @ W1[d].T + b1[d]; g = gelu(h); out[:, d] = g @ W2[d] + b2[d]
  x: [B=512, D=2048, M=128], W1: [D, H=128, M], b1: [D, H], W2: [D, H], b2: [D]

Strategy:
  - Shard on D: each of 8 cores owns D_LOC = 256 independent per-neuron MLPs.
  - Host pre-transposes so every DMA is contiguous:
      xT[d, m, b], W1T[d, m, h], W2T[h, d], b1T[h, d]
  - Inputs cast to fp16 on host (PSUM accumulation stays fp32): 4x PE stream
    rate vs fp32 and half the DMA bytes, with ~1e-3 worst-case output error.
  - Per quad of 4 d's on-chip (contraction dims on partitions):
      psum1[H, 2B] = W1T[d].T @ xT[d]          (N=512 matmuls, 2 per psum tile)
      g[H, 4B]     = gelu(psum1 + b1[d])       (ScalarE, exact-erf Gelu)
      psum2[128,B] = 4 packed matmuls W2T[:,d].T @ g_d, tile_position=(0,32j)
                     -> rows {0,32,64,96}
      o_sb         = DVE full-tile copy of psum2 (only 4 rows carry data)
      outT[d:d+4] <- strided-partition DMA of o_sb rows {0,32,64,96}
  - b2 added on host (it is outside the nonlinearity).
"""

import numpy as np

B, D, M, H = 512, 2048, 128, 128
N_CORES = 8
D_LOC = D // N_CORES  # 256
QUAD = 4     # d's per MM2 packing group
PAIR = 2     # d's per psum1/ACT batch
SUPER = 16   # d's per super-group: one x DMA, one w1 DMA, one out DMA
# Within a super-group [D0, D0+16), quad c (c=0..3) handles d = D0 + 4j + c
# (j=0..3); MM2 j lands on psum row 32j, so out rows {D0..D0+15} are exactly
# o_sb[0::32, c, :] in (row, quad, b) iteration order -> single strided DMA.

PRECISION = "fp16"  # "fp16" | "fp32"

_NC_CACHE = {}


def build_nc(bias_mode: bool, prec: str = PRECISION, reps: int = 1):
    """Build + compile the Bass module (shared SPMD program for all 8 cores).

    reps>1 replicates the whole body (same in/out DRAM) for benchmarking:
    one NEFF execution then runs the kernel `reps` times back-to-back."""
    key = (bias_mode, prec, reps)
    if key in _NC_CACHE:
        return _NC_CACHE[key]

    import concourse.bacc as bacc
    import concourse.mybir as mybir
    import concourse.tile as tile

    f32 = mybir.dt.float32
    dt = f32 if prec == "fp32" else mybir.dt.float16
    GELU = mybir.ActivationFunctionType.Gelu

    nc = bacc.Bacc("TRN2", target_bir_lowering=False, debug=False, num_devices=N_CORES)
    xT = nc.dram_tensor("xT", [D_LOC, M, B], dt, kind="ExternalInput").ap()
    w1T = nc.dram_tensor("w1T", [D_LOC, M, H], dt, kind="ExternalInput").ap()
    w2T = nc.dram_tensor("w2T", [H, D_LOC], dt, kind="ExternalInput").ap()
    b1T = nc.dram_tensor("b1T", [H, D_LOC], f32, kind="ExternalInput").ap()
    outT = nc.dram_tensor("outT", [D_LOC, B], f32, kind="ExternalOutput").ap()

    with (
        tile.TileContext(nc) as tc,
        tc.tile_pool(name="singles", bufs=1) as singles,
        tc.tile_pool(name="xp", bufs=3) as xp,
        tc.tile_pool(name="wp", bufs=2) as wp,
        tc.tile_pool(name="gp", bufs=4) as gp,
        tc.tile_pool(name="op", bufs=4) as op_pool,
        tc.tile_pool(name="ps1", bufs=2, space="PSUM") as ps1,
        tc.tile_pool(name="ps2", bufs=2, space="PSUM") as ps2,
    ):
        w2_sb = singles.tile([H, D_LOC], dt)
        nc.sync.dma_start(out=w2_sb[:], in_=w2T[:])
        b1_sb = None
        if bias_mode:
            b1_sb = singles.tile([H, D_LOC], f32)
            nc.sync.dma_start(out=b1_sb[:], in_=b1T[:])

        for _rep in range(reps):
            _body_loop(nc, tc, bias_mode, dt, f32, GELU,
                       xT, w1T, outT, w2_sb, b1_sb,
                       xp, wp, gp, op_pool, ps1, ps2)

    nc.compile()
    _NC_CACHE[key] = nc
    return nc


def _body_loop(nc, tc, bias_mode, dt, f32, GELU, xT, w1T, outT, w2_sb, b1_sb,
               xp, wp, gp, op_pool, ps1, ps2):
        for sg in range(D_LOC // SUPER):
            D0 = sg * SUPER
            x_sb = xp.tile([M, SUPER, B], dt)
            nc.sync.dma_start(
                out=x_sb[:],
                in_=xT[D0 : D0 + SUPER].rearrange("d m b -> m d b"),
            )
            w1_sb = wp.tile([M, SUPER, H], dt)
            nc.scalar.dma_start(
                out=w1_sb[:],
                in_=w1T[D0 : D0 + SUPER].rearrange("d m h -> m d h"),
            )
            o_sb = op_pool.tile([128, SUPER // QUAD, B], f32)
            for c in range(SUPER // QUAD):
                # quad c handles d = D0 + 4j + c, j = 0..3
                g_sb = gp.tile([H, QUAD * B], dt)
                for pr in range(QUAD // PAIR):
                    p1 = ps1.tile([H, PAIR * B], f32)
                    for j in range(PAIR):
                        jj = pr * PAIR + j
                        nc.tensor.matmul(
                            p1[:, j * B : (j + 1) * B],
                            lhsT=w1_sb[:, 4 * jj + c, :],
                            rhs=x_sb[:, 4 * jj + c, :],
                            start=True,
                            stop=True,
                        )
                    if bias_mode:
                        for j in range(PAIR):
                            jj = pr * PAIR + j
                            dd = D0 + 4 * jj + c
                            nc.scalar.activation(
                                g_sb[:, jj * B : (jj + 1) * B],
                                p1[:, j * B : (j + 1) * B],
                                GELU,
                                bias=b1_sb[:, dd : dd + 1],
                            )
                    else:
                        nc.scalar.activation(
                            g_sb[:, pr * PAIR * B : (pr + 1) * PAIR * B], p1[:], GELU
                        )
                p2 = ps2.tile([128, B], f32)
                for j in range(QUAD):
                    dd = D0 + 4 * j + c
                    nc.tensor.matmul(
                        p2[32 * j : 32 * j + 1, :],
                        lhsT=w2_sb[:, dd : dd + 1],
                        rhs=g_sb[:, j * B : (j + 1) * B],
                        start=True,
                        stop=True,
                        tile_position=(0, 32 * j),
                    )
                nc.vector.tensor_copy(o_sb[:, c, :], p2[:])
            nc.gpsimd.dma_start(
                out=outT[D0 : D0 + SUPER, :], in_=o_sb[0::32, :, :]
            )


def prepare_in_maps(x, W1, b1, W2, prec: str = PRECISION):
    """Host-side shard + transpose. Returns list of 8 per-core input dicts."""
    np_dt = np.float32 if prec == "fp32" else np.float16
    x = np.asarray(x, dtype=np.float32)
    W1 = np.asarray(W1, dtype=np.float32)
    b1 = np.asarray(b1, dtype=np.float32)
    W2 = np.asarray(W2, dtype=np.float32)

    in_maps = []
    for k in range(N_CORES):
        sl = slice(k * D_LOC, (k + 1) * D_LOC)
        # [B, D_LOC, M] -> [D_LOC, M, B]; see kernel docstring for why.
        xT_k = np.ascontiguousarray(x[:, sl, :].transpose(1, 2, 0), dtype=np_dt)
        w1T_k = np.ascontiguousarray(W1[sl].transpose(0, 2, 1), dtype=np_dt)
        w2T_k = np.ascontiguousarray(W2[sl].T, dtype=np_dt)
        b1T_k = np.ascontiguousarray(b1[sl].T, dtype=np.float32)
        in_maps.append({"xT": xT_k, "w1T": w1T_k, "w2T": w2T_k, "b1T": b1T_k})
    return in_maps


def assemble_output(results, b2):
    outT_full = np.concatenate([r["outT"] for r in results], axis=0)  # [D, B]
    out = outT_full.T  # [B, D]
    b2 = np.asarray(b2, dtype=np.float32)
    if np.any(b2):
        out = out + b2[None, :]
    return np.ascontiguousarray(out)


def kernel(pre_activation_history, W1, b1, W2, b2):
    from concourse.bass_utils import run_bass_kernel_spmd

    b1 = np.asarray(b1, dtype=np.float32)
    bias_mode = bool(np.any(b1))
    nc = build_nc(bias_mode)
    in_maps = prepare_in_maps(pre_activation_history, W1, b1, W2)
    res = run_bass_kernel_spmd(nc, in_maps, core_ids=list(range(N_CORES)))
    return assemble_output(res.results, b2)
